# revision 3
# baseline (speedup 1.0000x reference)
"""Trainium2 Bass kernel for nn_ModelSimplest (4D conv -> relu -> linear -> sigmoid).

Strategy: pure data parallel over batch (1024 -> 8 cores x 128).
The 4D conv is mapped onto TensorE matmuls:
  - contraction over the (k,l) input plane (324 values, split into 3 chunks
    of 108 partitions), expressed as a 2D-Toeplitz stationary matrix
    [108 x (3ch*6k'*6l' = 108)] built on the host from W4,
  - accumulation over the 169 (a,b) kernel offsets of the first two spatial
    dims (and the 3 chunks) in PSUM,
  - the moving operand streams (batch, j-window) columns: N = 64*6 = 384.
Epilogue fused on-chip: bias + relu (ScalarE, PSUM->SBUF bf16), then the
Linear(3888->1) as 36 accumulating [108x1] matmuls per batch tile, then
bias + sigmoid (ScalarE) and DMA out.

All data layout transforms / dtype casts are done host-side in numpy.
"""
import sys
from contextlib import ExitStack

import numpy as np

sys.path.insert(0, "/opt/trn_rl_repo")

from concourse import bacc, bass, mybir, tile  # noqa: E402
from concourse.bass_utils import run_bass_kernel_spmd  # noqa: E402

KK = 13      # conv kernel size per dim
S_IN = 18
S_OUT = 6
N_CORES = 8
B_TOTAL = 1024
B_CORE = B_TOTAL // N_CORES          # 128
B_SUB = 64                            # batch subtile per PSUM pass
N_SUB = B_CORE // B_SUB               # 2
NCH = 3
NPART = 108                           # partitions per contraction chunk
NM = NCH * S_OUT * S_OUT              # 108 output features per matmul
NCHUNK = 3                            # 324 = 3 * 108

F32 = mybir.dt.float32
BF16 = mybir.dt.bfloat16

_CACHE = {}


def _build_nc():
    nc = bacc.Bacc(None, target_bir_lowering=False)

    xp = nc.dram_tensor("xp", [NCHUNK, NPART, B_CORE, 324], BF16,
                        kind="ExternalInput")
    tw = nc.dram_tensor("tw", [NCHUNK, KK, NPART, KK, NM], BF16,
                        kind="ExternalInput")
    wl = nc.dram_tensor("wl", [NPART, S_OUT * S_OUT], BF16,
                        kind="ExternalInput")
    bias4 = nc.dram_tensor("bias4", [NPART, 1], F32, kind="ExternalInput")
    blin = nc.dram_tensor("blin", [1, 1], F32, kind="ExternalInput")
    out = nc.dram_tensor("out", [1, B_CORE], F32, kind="ExternalOutput")

    with tile.TileContext(nc) as tc, ExitStack() as ctx:
        cpool = ctx.enter_context(tc.tile_pool(name="consts", bufs=1))
        wl_sb = cpool.tile([NPART, S_OUT * S_OUT], BF16)
        bias_sb = cpool.tile([NPART, 1], F32)
        blin_sb = cpool.tile([1, 1], F32)
        nc.sync.dma_start(wl_sb[:], wl[:])
        nc.sync.dma_start(bias_sb[:], bias4[:])
        nc.sync.dma_start(blin_sb[:], blin[:])

        xpool = ctx.enter_context(tc.tile_pool(name="xs", bufs=2))
        twpool = ctx.enter_context(tc.tile_pool(name="tws", bufs=3))
        pspool = ctx.enter_context(
            tc.tile_pool(name="ps", bufs=1, space=bass.MemorySpace.PSUM))
        hpool = ctx.enter_context(tc.tile_pool(name="hs", bufs=1))
        opool = ctx.enter_context(tc.tile_pool(name="outs", bufs=2))

        for t in range(N_SUB):
            ps = [
                pspool.tile([NM, B_SUB, S_OUT], F32, tag=f"ps{i}", name=f"ps{i}_{t}")
                for i in range(S_OUT)
            ]
            for c in range(NCHUNK):
                xt = xpool.tile([NPART, B_SUB, 324], BF16, tag="xt")
                nc.sync.dma_start(
                    xt[:], xp[c, :, t * B_SUB:(t + 1) * B_SUB, :])
                for a in range(KK):
                    twt = twpool.tile([NPART, KK, NM], BF16, tag="twt")
                    nc.sync.dma_start(twt[:], tw[c, a])
                    first = (c == 0 and a == 0)
                    last = (c == NCHUNK - 1 and a == KK - 1)
                    for boff in range(KK):
                        lhsT = twt[:, boff, :]
                        for i in range(S_OUT):
                            # rhs: [108, b_sub, 6] window over jb
                            ia = i + a
                            rhs = xt[:, :, ia * S_IN + boff:
                                     ia * S_IN + boff + S_OUT]
                            nc.tensor.matmul(
                                ps[i][:],
                                lhsT,
                                rhs,
                                start=(first and boff == 0),
                                stop=(last and boff == KK - 1),
                            )
            # epilogue: bias+relu -> h ; linear -> logit psum
            lg = pspool.tile([1, B_SUB], F32, tag="lg", name=f"lg_{t}")
            for i in range(S_OUT):
                h = hpool.tile([NM, B_SUB, S_OUT], BF16, tag=f"h{i}")
                nc.scalar.activation(
                    h[:], ps[i][:],
                    mybir.ActivationFunctionType.Relu,
                    bias=bias_sb[:],
                )
                for j in range(S_OUT):
                    nc.tensor.matmul(
                        lg[:],
                        wl_sb[:, i * S_OUT + j:i * S_OUT + j + 1],
                        h[:, :, j],
                        start=(i == 0 and j == 0),
                        stop=(i == S_OUT - 1 and j == S_OUT - 1),
                    )
            ot = opool.tile([1, B_SUB], F32, tag="ot")
            nc.scalar.activation(
                ot[:], lg[:],
                mybir.ActivationFunctionType.Sigmoid,
                bias=blin_sb[:],
            )
            nc.sync.dma_start(out[:, t * B_SUB:(t + 1) * B_SUB], ot[:])

    nc.compile()
    return nc


def _prep_inputs(x, W4, b4, Wlin, blin):
    """Host-side layout transforms. Returns the shared (weight) arrays and
    the per-core x shards."""
    B = x.shape[0]
    # x_prep[c, p, b, ia*18+jb] with c*108+p = k_in*18+l_in
    xt = np.ascontiguousarray(
        x[:, 0].transpose(3, 4, 0, 1, 2)).reshape(324, B, 324)
    x_prep = xt.reshape(NCHUNK, NPART, B, 324).astype(np.float32)
    x_prep = x_prep.astype(jnp_bf16)

    # T_flat[kl, a, boff, m]
    T_flat = np.zeros((324, KK, KK, NM), np.float32)
    kl = np.arange(324)
    k_in_v = kl // S_IN
    l_in_v = kl % S_IN
    W4t = W4[:, 0].transpose(0, 3, 4, 1, 2)  # [ch, dk, dl, a, boff]
    for ch in range(NCH):
        for kp in range(S_OUT):
            for lp in range(S_OUT):
                m = ch * 36 + kp * 6 + lp
                dk = k_in_v - kp
                dl = l_in_v - lp
                valid = (dk >= 0) & (dk < KK) & (dl >= 0) & (dl < KK)
                T_flat[valid, :, :, m] = W4t[ch, dk[valid], dl[valid]]
    # -> [c, a, p, boff, m]
    T_all = np.ascontiguousarray(
        T_flat.reshape(NCHUNK, NPART, KK, KK, NM).transpose(0, 2, 1, 3, 4))
    tw_np = T_all.astype(jnp_bf16)  # [c, a, p, boff, m]

    # wl[m, i*6+j] = Wlin[0, ch*1296 + i*216 + j*36 + (m%36)]
    m_idx = np.arange(NPART)
    ch_idx = m_idx // 36
    rem = m_idx % 36
    i_idx = np.arange(S_OUT)
    j_idx = np.arange(S_OUT)
    feat = (ch_idx[:, None, None] * 1296 + i_idx[None, :, None] * 216
            + j_idx[None, None, :] * 36 + rem[:, None, None])
    wl_np = Wlin[0, feat].reshape(NPART, S_OUT * S_OUT).astype(jnp_bf16)

    bias4_np = np.ascontiguousarray(
        b4[m_idx // 36].astype(np.float32).reshape(NPART, 1))
    blin_np = np.asarray(blin, np.float32).reshape(1, 1)
    return x_prep, tw_np, wl_np, bias4_np, blin_np


try:
    import ml_dtypes
    jnp_bf16 = ml_dtypes.bfloat16
except ImportError:  # pragma: no cover
    import jax.numpy as jnp
    jnp_bf16 = jnp.bfloat16


def kernel(x, W4, b4, Wlin, blin, _profile=False):
    x = np.asarray(x)
    W4 = np.asarray(W4)
    b4 = np.asarray(b4)
    Wlin = np.asarray(Wlin)
    blin = np.asarray(blin)

    x_prep, tw_np, wl_np, bias4_np, blin_np = _prep_inputs(
        x, W4, b4, Wlin, blin)

    if "nc" not in _CACHE:
        _CACHE["nc"] = _build_nc()
    nc = _CACHE["nc"]

    in_maps = []
    for core in range(N_CORES):
        b0 = core * B_CORE
        in_maps.append({
            "xp": np.ascontiguousarray(x_prep[:, :, b0:b0 + B_CORE, :]),
            "tw": tw_np,
            "wl": wl_np,
            "bias4": bias4_np,
            "blin": blin_np,
        })

    res = run_bass_kernel_spmd(
        nc, in_maps, core_ids=list(range(N_CORES)), trace=_profile)
    outs = [res.results[i]["out"].reshape(B_CORE) for i in range(N_CORES)]
    full = np.concatenate(outs).reshape(B_TOTAL, 1).astype(np.float32)
    if _profile:
        return full, res
    return full


# revision 4
# speedup vs baseline: 1.3391x; 1.3391x over previous
"""Trainium2 Bass kernel for nn_ModelSimplest (4D conv -> relu -> linear -> sigmoid).

Strategy: pure data parallel over batch (1024 -> 8 cores x 128).
The 4D conv is mapped onto TensorE matmuls:
  - contraction over the (k,l) input plane (324 values, split into 3 chunks
    of 108 partitions), expressed as a 2D-Toeplitz stationary matrix
    [108 x (3ch*6k'*6l' = 108)] built on the host from W4,
  - accumulation over the 169 (a,b) kernel offsets of the first two spatial
    dims (and the 3 chunks) in PSUM,
  - the moving operand streams (batch, j-window) columns: N = 64*6 = 384.
Epilogue fused on-chip: bias + relu (ScalarE, PSUM->SBUF bf16), then the
Linear(3888->1) as 36 accumulating [108x1] matmuls per batch tile, then
bias + sigmoid (ScalarE) and DMA out.

All data layout transforms / dtype casts are done host-side in numpy.
"""
import sys
from contextlib import ExitStack

import numpy as np

sys.path.insert(0, "/opt/trn_rl_repo")

from concourse import bacc, bass, mybir, tile  # noqa: E402
from concourse.bass_utils import run_bass_kernel_spmd  # noqa: E402

KK = 13      # conv kernel size per dim
S_IN = 18
S_OUT = 6
N_CORES = 8
B_TOTAL = 1024
B_CORE = B_TOTAL // N_CORES          # 128
B_SUB = 64                            # batch subtile per PSUM pass
N_SUB = B_CORE // B_SUB               # 2
NCH = 3
NPART = 108                           # partitions per contraction chunk
NM = NCH * S_OUT * S_OUT              # 108 output features per matmul
NCHUNK = 3                            # 324 = 3 * 108

F32 = mybir.dt.float32
BF16 = mybir.dt.bfloat16

_CACHE = {}


def _build_nc():
    nc = bacc.Bacc(None, target_bir_lowering=False)

    xp = nc.dram_tensor("xp", [NCHUNK, N_SUB, NPART, S_IN, S_IN, B_SUB],
                        BF16, kind="ExternalInput")
    tw = nc.dram_tensor("tw", [NCHUNK, KK, NPART, KK, NM], BF16,
                        kind="ExternalInput")
    wl = nc.dram_tensor("wl", [NPART, S_OUT * S_OUT], BF16,
                        kind="ExternalInput")
    bias4 = nc.dram_tensor("bias4", [NPART, 1], F32, kind="ExternalInput")
    blin = nc.dram_tensor("blin", [1, 1], F32, kind="ExternalInput")
    out = nc.dram_tensor("out", [1, B_CORE], F32, kind="ExternalOutput")

    with tile.TileContext(nc) as tc, ExitStack() as ctx:
        cpool = ctx.enter_context(tc.tile_pool(name="consts", bufs=1))
        wl_sb = cpool.tile([NPART, S_OUT * S_OUT], BF16)
        bias_sb = cpool.tile([NPART, 1], F32)
        blin_sb = cpool.tile([1, 1], F32)
        nc.sync.dma_start(wl_sb[:], wl[:])
        nc.sync.dma_start(bias_sb[:], bias4[:])
        nc.sync.dma_start(blin_sb[:], blin[:])

        xpool = ctx.enter_context(tc.tile_pool(name="xs", bufs=2))
        twpool = ctx.enter_context(tc.tile_pool(name="tws", bufs=3))
        pspool = ctx.enter_context(
            tc.tile_pool(name="ps", bufs=1, space=bass.MemorySpace.PSUM))
        hpool = ctx.enter_context(tc.tile_pool(name="hs", bufs=1))
        opool = ctx.enter_context(tc.tile_pool(name="outs", bufs=2))

        for t in range(N_SUB):
            ps = [
                pspool.tile([NM, S_OUT, B_SUB], F32, tag=f"ps{i}", name=f"ps{i}_{t}")
                for i in range(S_OUT)
            ]
            for c in range(NCHUNK):
                xt = xpool.tile([NPART, S_IN, S_IN, B_SUB], BF16, tag="xt")
                nc.sync.dma_start(xt[:], xp[c, t])
                for a in range(KK):
                    twt = twpool.tile([NPART, KK, NM], BF16, tag="twt")
                    nc.sync.dma_start(twt[:], tw[c, a])
                    first = (c == 0 and a == 0)
                    last = (c == NCHUNK - 1 and a == KK - 1)
                    for boff in range(KK):
                        lhsT = twt[:, boff, :]
                        for i in range(S_OUT):
                            # rhs: [108, 6, b_sub] (jb window, batch inner)
                            ia = i + a
                            rhs = xt[:, ia, boff:boff + S_OUT, :]
                            nc.tensor.matmul(
                                ps[i][:],
                                lhsT,
                                rhs,
                                start=(first and boff == 0),
                                stop=(last and boff == KK - 1),
                            )
            # epilogue: bias+relu -> h ; linear -> logit psum
            lg = pspool.tile([1, B_SUB], F32, tag="lg", name=f"lg_{t}")
            for i in range(S_OUT):
                h = hpool.tile([NM, S_OUT, B_SUB], BF16, tag=f"h{i}")
                nc.scalar.activation(
                    h[:], ps[i][:],
                    mybir.ActivationFunctionType.Relu,
                    bias=bias_sb[:],
                )
                for j in range(S_OUT):
                    nc.tensor.matmul(
                        lg[:],
                        wl_sb[:, i * S_OUT + j:i * S_OUT + j + 1],
                        h[:, j, :],
                        start=(i == 0 and j == 0),
                        stop=(i == S_OUT - 1 and j == S_OUT - 1),
                    )
            ot = opool.tile([1, B_SUB], F32, tag="ot")
            nc.scalar.activation(
                ot[:], lg[:],
                mybir.ActivationFunctionType.Sigmoid,
                bias=blin_sb[:],
            )
            nc.sync.dma_start(out[:, t * B_SUB:(t + 1) * B_SUB], ot[:])

    nc.compile()
    return nc


def _prep_inputs(x, W4, b4, Wlin, blin):
    """Host-side layout transforms. Returns the shared (weight) arrays and
    the per-core x shards."""
    B = x.shape[0]
    # x_prep[c*108+p = k*18+l][ia][jb][b], bf16
    xt = np.ascontiguousarray(
        x[:, 0].transpose(3, 4, 1, 2, 0)).astype(jnp_bf16)
    x_prep = xt.reshape(NCHUNK, NPART, S_IN, S_IN, B)

    # T_flat[kl, a, boff, m]
    T_flat = np.zeros((324, KK, KK, NM), np.float32)
    kl = np.arange(324)
    k_in_v = kl // S_IN
    l_in_v = kl % S_IN
    W4t = W4[:, 0].transpose(0, 3, 4, 1, 2)  # [ch, dk, dl, a, boff]
    for ch in range(NCH):
        for kp in range(S_OUT):
            for lp in range(S_OUT):
                m = ch * 36 + kp * 6 + lp
                dk = k_in_v - kp
                dl = l_in_v - lp
                valid = (dk >= 0) & (dk < KK) & (dl >= 0) & (dl < KK)
                T_flat[valid, :, :, m] = W4t[ch, dk[valid], dl[valid]]
    # -> [c, a, p, boff, m]
    T_all = np.ascontiguousarray(
        T_flat.reshape(NCHUNK, NPART, KK, KK, NM).transpose(0, 2, 1, 3, 4))
    tw_np = T_all.astype(jnp_bf16)  # [c, a, p, boff, m]

    # wl[m, i*6+j] = Wlin[0, ch*1296 + i*216 + j*36 + (m%36)]
    m_idx = np.arange(NPART)
    ch_idx = m_idx // 36
    rem = m_idx % 36
    i_idx = np.arange(S_OUT)
    j_idx = np.arange(S_OUT)
    feat = (ch_idx[:, None, None] * 1296 + i_idx[None, :, None] * 216
            + j_idx[None, None, :] * 36 + rem[:, None, None])
    wl_np = Wlin[0, feat].reshape(NPART, S_OUT * S_OUT).astype(jnp_bf16)

    bias4_np = np.ascontiguousarray(
        b4[m_idx // 36].astype(np.float32).reshape(NPART, 1))
    blin_np = np.asarray(blin, np.float32).reshape(1, 1)
    return x_prep, tw_np, wl_np, bias4_np, blin_np


try:
    import ml_dtypes
    jnp_bf16 = ml_dtypes.bfloat16
except ImportError:  # pragma: no cover
    import jax.numpy as jnp
    jnp_bf16 = jnp.bfloat16


def kernel(x, W4, b4, Wlin, blin, _profile=False):
    x = np.asarray(x)
    W4 = np.asarray(W4)
    b4 = np.asarray(b4)
    Wlin = np.asarray(Wlin)
    blin = np.asarray(blin)

    x_prep, tw_np, wl_np, bias4_np, blin_np = _prep_inputs(
        x, W4, b4, Wlin, blin)

    if "nc" not in _CACHE:
        _CACHE["nc"] = _build_nc()
    nc = _CACHE["nc"]

    in_maps = []
    for core in range(N_CORES):
        b0 = core * B_CORE
        shard = x_prep[:, :, :, :, b0:b0 + B_CORE]
        shard = shard.reshape(NCHUNK, NPART, S_IN, S_IN, N_SUB, B_SUB)
        shard = np.ascontiguousarray(shard.transpose(0, 4, 1, 2, 3, 5))
        in_maps.append({
            "xp": shard,
            "tw": tw_np,
            "wl": wl_np,
            "bias4": bias4_np,
            "blin": blin_np,
        })

    res = run_bass_kernel_spmd(
        nc, in_maps, core_ids=list(range(N_CORES)), trace=_profile)
    outs = [res.results[i]["out"].reshape(B_CORE) for i in range(N_CORES)]
    full = np.concatenate(outs).reshape(B_TOTAL, 1).astype(np.float32)
    if _profile:
        return full, res
    return full


# revision 5
# speedup vs baseline: 1.5343x; 1.1458x over previous
"""Trainium2 Bass kernel for nn_ModelSimplest (4D conv -> relu -> linear -> sigmoid).

Strategy: pure data parallel over batch (1024 -> 8 cores x 128).
The 4D conv is mapped onto TensorE matmuls:
  - contraction over the (k,l) input plane (324 values, split into 3 chunks
    of 108 partitions), expressed as a 2D-Toeplitz stationary matrix
    [108 x (3ch*6k'*6l' = 108)] built on the host from W4,
  - accumulation over the 169 (a,b) kernel offsets of the first two spatial
    dims (and the 3 chunks) in PSUM,
  - the moving operand streams (batch, j-window) columns: N = 64*6 = 384.
Epilogue fused on-chip: bias + relu (ScalarE, PSUM->SBUF bf16), then the
Linear(3888->1) as 36 accumulating [108x1] matmuls per batch tile, then
bias + sigmoid (ScalarE) and DMA out.

All data layout transforms / dtype casts are done host-side in numpy.
"""
import sys
from contextlib import ExitStack

import numpy as np

sys.path.insert(0, "/opt/trn_rl_repo")

from concourse import bacc, bass, mybir, tile  # noqa: E402
from concourse.bass_utils import run_bass_kernel_spmd  # noqa: E402

KK = 13      # conv kernel size per dim
S_IN = 18
S_OUT = 6
N_CORES = 8
B_TOTAL = 1024
B_CORE = B_TOTAL // N_CORES          # 128
B_SUB = 64                            # batch subtile per PSUM pass
N_SUB = B_CORE // B_SUB               # 2
NCH = 3
NPART = 108                           # partitions per contraction chunk
NM = NCH * S_OUT * S_OUT              # 108 output features per matmul
NCHUNK = 3                            # 324 = 3 * 108

F32 = mybir.dt.float32
BF16 = mybir.dt.bfloat16

_CACHE = {}


def _build_nc():
    nc = bacc.Bacc(None, target_bir_lowering=False)

    xp = nc.dram_tensor("xp", [NCHUNK, N_SUB, NPART, S_IN, S_IN, B_SUB],
                        BF16, kind="ExternalInput")
    tw = nc.dram_tensor("tw", [NCHUNK, KK, NPART, KK, NM], BF16,
                        kind="ExternalInput")
    wl = nc.dram_tensor("wl", [NPART, S_OUT * S_OUT], BF16,
                        kind="ExternalInput")
    bias4 = nc.dram_tensor("bias4", [NPART, 1], F32, kind="ExternalInput")
    blin = nc.dram_tensor("blin", [1, 1], F32, kind="ExternalInput")
    out = nc.dram_tensor("out", [1, B_CORE], F32, kind="ExternalOutput")

    with tile.TileContext(nc) as tc, ExitStack() as ctx:
        cpool = ctx.enter_context(tc.tile_pool(name="consts", bufs=1))
        wl_sb = cpool.tile([NPART, S_OUT * S_OUT], BF16)
        bias_sb = cpool.tile([NPART, 1], F32)
        blin_sb = cpool.tile([1, 1], F32)
        nc.sync.dma_start(wl_sb[:], wl[:])
        nc.sync.dma_start(bias_sb[:], bias4[:])
        nc.sync.dma_start(blin_sb[:], blin[:])

        xpool = ctx.enter_context(tc.tile_pool(name="xs", bufs=2))
        twpool = ctx.enter_context(tc.tile_pool(name="tws", bufs=1))
        tw_tiles = {}
        for c in range(NCHUNK):
            for a in range(KK):
                twt = twpool.tile([NPART, KK, NM], BF16, tag=f"tw{c}_{a}",
                                  name=f"tw{c}_{a}")
                nc.sync.dma_start(twt[:], tw[c, a])
                tw_tiles[(c, a)] = twt
        pspool = ctx.enter_context(
            tc.tile_pool(name="ps", bufs=1, space=bass.MemorySpace.PSUM))
        hpool = ctx.enter_context(tc.tile_pool(name="hs", bufs=1))
        opool = ctx.enter_context(tc.tile_pool(name="outs", bufs=2))

        for t in range(N_SUB):
            ps = [
                pspool.tile([NM, S_OUT, B_SUB], F32, tag=f"ps{i}", name=f"ps{i}_{t}")
                for i in range(S_OUT)
            ]
            for c in range(NCHUNK):
                xt = xpool.tile([NPART, S_IN, S_IN, B_SUB], BF16, tag="xt")
                nc.sync.dma_start(xt[:], xp[c, t])
                for a in range(KK):
                    twt = tw_tiles[(c, a)]
                    first = (c == 0 and a == 0)
                    last = (c == NCHUNK - 1 and a == KK - 1)
                    for boff in range(KK):
                        lhsT = twt[:, boff, :]
                        for i in range(S_OUT):
                            # rhs: [108, 6, b_sub] (jb window, batch inner)
                            ia = i + a
                            rhs = xt[:, ia, boff:boff + S_OUT, :]
                            nc.tensor.matmul(
                                ps[i][:],
                                lhsT,
                                rhs,
                                start=(first and boff == 0),
                                stop=(last and boff == KK - 1),
                            )
            # epilogue: bias+relu -> h ; linear -> logit psum
            lg = pspool.tile([1, B_SUB], F32, tag="lg", name=f"lg_{t}")
            for i in range(S_OUT):
                h = hpool.tile([NM, S_OUT, B_SUB], BF16, tag=f"h{i}")
                nc.scalar.activation(
                    h[:], ps[i][:],
                    mybir.ActivationFunctionType.Relu,
                    bias=bias_sb[:],
                )
                for j in range(S_OUT):
                    nc.tensor.matmul(
                        lg[:],
                        wl_sb[:, i * S_OUT + j:i * S_OUT + j + 1],
                        h[:, j, :],
                        start=(i == 0 and j == 0),
                        stop=(i == S_OUT - 1 and j == S_OUT - 1),
                    )
            ot = opool.tile([1, B_SUB], F32, tag="ot")
            nc.scalar.activation(
                ot[:], lg[:],
                mybir.ActivationFunctionType.Sigmoid,
                bias=blin_sb[:],
            )
            nc.sync.dma_start(out[:, t * B_SUB:(t + 1) * B_SUB], ot[:])

    nc.compile()
    return nc


def _prep_inputs(x, W4, b4, Wlin, blin):
    """Host-side layout transforms. Returns the shared (weight) arrays and
    the per-core x shards."""
    B = x.shape[0]
    # x_prep[c*108+p = k*18+l][ia][jb][b], bf16
    xt = np.ascontiguousarray(
        x[:, 0].transpose(3, 4, 1, 2, 0)).astype(jnp_bf16)
    x_prep = xt.reshape(NCHUNK, NPART, S_IN, S_IN, B)

    # T_flat[kl, a, boff, m]
    T_flat = np.zeros((324, KK, KK, NM), np.float32)
    kl = np.arange(324)
    k_in_v = kl // S_IN
    l_in_v = kl % S_IN
    W4t = W4[:, 0].transpose(0, 3, 4, 1, 2)  # [ch, dk, dl, a, boff]
    for ch in range(NCH):
        for kp in range(S_OUT):
            for lp in range(S_OUT):
                m = ch * 36 + kp * 6 + lp
                dk = k_in_v - kp
                dl = l_in_v - lp
                valid = (dk >= 0) & (dk < KK) & (dl >= 0) & (dl < KK)
                T_flat[valid, :, :, m] = W4t[ch, dk[valid], dl[valid]]
    # -> [c, a, p, boff, m]
    T_all = np.ascontiguousarray(
        T_flat.reshape(NCHUNK, NPART, KK, KK, NM).transpose(0, 2, 1, 3, 4))
    tw_np = T_all.astype(jnp_bf16)  # [c, a, p, boff, m]

    # wl[m, i*6+j] = Wlin[0, ch*1296 + i*216 + j*36 + (m%36)]
    m_idx = np.arange(NPART)
    ch_idx = m_idx // 36
    rem = m_idx % 36
    i_idx = np.arange(S_OUT)
    j_idx = np.arange(S_OUT)
    feat = (ch_idx[:, None, None] * 1296 + i_idx[None, :, None] * 216
            + j_idx[None, None, :] * 36 + rem[:, None, None])
    wl_np = Wlin[0, feat].reshape(NPART, S_OUT * S_OUT).astype(jnp_bf16)

    bias4_np = np.ascontiguousarray(
        b4[m_idx // 36].astype(np.float32).reshape(NPART, 1))
    blin_np = np.asarray(blin, np.float32).reshape(1, 1)
    return x_prep, tw_np, wl_np, bias4_np, blin_np


try:
    import ml_dtypes
    jnp_bf16 = ml_dtypes.bfloat16
except ImportError:  # pragma: no cover
    import jax.numpy as jnp
    jnp_bf16 = jnp.bfloat16


def kernel(x, W4, b4, Wlin, blin, _profile=False):
    x = np.asarray(x)
    W4 = np.asarray(W4)
    b4 = np.asarray(b4)
    Wlin = np.asarray(Wlin)
    blin = np.asarray(blin)

    x_prep, tw_np, wl_np, bias4_np, blin_np = _prep_inputs(
        x, W4, b4, Wlin, blin)

    if "nc" not in _CACHE:
        _CACHE["nc"] = _build_nc()
    nc = _CACHE["nc"]

    in_maps = []
    for core in range(N_CORES):
        b0 = core * B_CORE
        shard = x_prep[:, :, :, :, b0:b0 + B_CORE]
        shard = shard.reshape(NCHUNK, NPART, S_IN, S_IN, N_SUB, B_SUB)
        shard = np.ascontiguousarray(shard.transpose(0, 4, 1, 2, 3, 5))
        in_maps.append({
            "xp": shard,
            "tw": tw_np,
            "wl": wl_np,
            "bias4": bias4_np,
            "blin": blin_np,
        })

    res = run_bass_kernel_spmd(
        nc, in_maps, core_ids=list(range(N_CORES)), trace=_profile)
    outs = [res.results[i]["out"].reshape(B_CORE) for i in range(N_CORES)]
    full = np.concatenate(outs).reshape(B_TOTAL, 1).astype(np.float32)
    if _profile:
        return full, res
    return full
